# revision 9
# baseline (speedup 1.0000x reference)
"""MoE fused token-gen kernel for Trainium2, distributed over 8 NeuronCores.

Problem: 4 tokens, top-2 of 16 routed GLU experts (H=2048, I=1408) plus a
shared GLU expert (IS=5632), all f32 weights.

Strategy (expert-parallel dispatch, combine on host):
- Host computes the routing (softmax + top-2) in numpy only to decide WHICH
  expert weights to ship where (the dispatch).  The device recomputes the
  router, softmax and top-2 mask itself from the raw inputs, so all math that
  affects the output runs on device.
- The work is a flat list of 128-column "units": 11 units per selected routed
  expert (I=1408) and 44 units for the shared expert (IS=5632).  Units are
  balanced across the 8 cores; every core gets the same fixed capacity NU
  (padded with zero-scale duplicates).
- Weights are pre-sliced per core and cast to bf16 on host (memory-bound
  problem: halves HBM traffic; accumulation stays f32 in PSUM).
- Per unit u with columns c (and expert e): the device computes
  gT[c,4] = Wg[:,c].T @ x.T, uT likewise, h = silu(gT)*uT, scales h by the
  per-token affinity vector of e (zero for tokens that did not pick e,
  one for shared-expert units), and accumulates h.T @ Wd[c,:] into one
  [4,2048] PSUM accumulator shared by all units.
- Each core DMAs its [4,2048] partial; the host sums the 8 partials.
"""

import math
import numpy as np
import ml_dtypes

H = 2048
E = 16
K_TOP = 2
I_RT = 1408
I_SH = 5632
T = 4
NCORES = 8
P = 128
HT = H // P  # 16 h-tiles

BF16 = ml_dtypes.bfloat16

_BUILD_CACHE: dict[int, object] = {}
LAST_RESULT = None  # BassKernelResults of the most recent run (for test harness)


def _build_program(nu: int):
    """Build + compile the 8-core SPMD Bass program for `nu` units per core."""
    import concourse.bass as bass
    import concourse.bacc as bacc
    import concourse.mybir as mybir
    import concourse.tile as tile

    f32 = mybir.dt.float32
    bf16 = mybir.dt.bfloat16
    C = nu * P

    nc = bacc.Bacc(
        "TRN2",
        target_bir_lowering=False,
        debug=False,
        enable_asserts=False,
        num_devices=NCORES,
    )

    wg_d = nc.dram_tensor("wg", [HT, P, C], bf16, kind="ExternalInput").ap()
    wu_d = nc.dram_tensor("wu", [HT, P, C], bf16, kind="ExternalInput").ap()
    wd_d = nc.dram_tensor("wd", [C, H], bf16, kind="ExternalInput").ap()
    oh_d = nc.dram_tensor("oh", [E + 1, C], f32, kind="ExternalInput").ap()
    xt_d = nc.dram_tensor("xt", [P, HT, T], f32, kind="ExternalInput").ap()
    rwt_d = nc.dram_tensor("rwt", [P, HT, E], f32, kind="ExternalInput").ap()
    id4_d = nc.dram_tensor("id4", [T, T], f32, kind="ExternalInput").ap()
    out_d = nc.dram_tensor("out", [T, H], f32, kind="ExternalOutput").ap()

    AF = mybir.ActivationFunctionType
    ALU = mybir.AluOpType
    AX = mybir.AxisListType

    with tile.TileContext(nc) as tc:
        with (
            tc.tile_pool(name="const", bufs=1) as cpool,
            tc.tile_pool(name="wgp", bufs=1) as wgp,
            tc.tile_pool(name="wup", bufs=1) as wup,
            tc.tile_pool(name="wdp", bufs=4) as wdp,
            tc.tile_pool(name="small", bufs=8) as small,
            tc.tile_pool(name="pacc", bufs=1, space="PSUM") as pacc,
            tc.tile_pool(name="psmall", bufs=4, space="PSUM") as psmall,
        ):
            # ---- constant-ish loads ----
            xt_s = cpool.tile([P, HT, T], f32, tag="xt")
            nc.sync.dma_start(xt_s[:], xt_d[:])
            rwt_s = cpool.tile([P, HT, E], f32, tag="rwt")
            nc.sync.dma_start(rwt_s[:], rwt_d[:])
            oh_s = cpool.tile([E + 1, C], f32, tag="oh")
            nc.sync.dma_start(oh_s[:], oh_d[:])
            id4_s = cpool.tile([T, T], f32, tag="id4")
            nc.sync.dma_start(id4_s[:], id4_d[:])

            # big weight streams
            wg_t = []
            wu_t = []
            for ht in range(HT):
                wt = wgp.tile([P, C], bf16, tag=f"wg{ht}")
                nc.sync.dma_start(wt[:], wg_d[ht])
                wg_t.append(wt)
            for ht in range(HT):
                wt = wup.tile([P, C], bf16, tag=f"wu{ht}")
                nc.sync.dma_start(wt[:], wu_d[ht])
                wu_t.append(wt)

            # x cast to bf16 for the big matmuls
            xtb = cpool.tile([P, HT, T], bf16, tag="xtb")
            nc.vector.tensor_copy(xtb[:], xt_s[:])

            # ---- router: logits [4,16] = x @ Rw.T ----
            lg_ps = psmall.tile([T, E], f32, tag="ps")
            for ht in range(HT):
                nc.tensor.matmul(
                    lg_ps[:],
                    xt_s[:, ht, :],
                    rwt_s[:, ht, :],
                    start=(ht == 0),
                    stop=(ht == HT - 1),
                )
            # softmax over E (free axis)
            nmx = small.tile([T, 1], f32, tag="r1")
            nc.vector.tensor_reduce(nmx[:], lg_ps[:], axis=AX.X, op=ALU.max, negate=True)
            ex = small.tile([T, E], f32, tag="r2")
            nc.scalar.activation(ex[:], lg_ps[:], AF.Exp, bias=nmx[:])
            sm = small.tile([T, 1], f32, tag="r3")
            nc.vector.tensor_reduce(sm[:], ex[:], axis=AX.X, op=ALU.add)
            rc = small.tile([T, 1], f32, tag="r4")
            nc.vector.reciprocal(rc[:], sm[:])
            aff = small.tile([T, E], f32, tag="r5")
            nc.vector.tensor_scalar_mul(aff[:], ex[:], rc[:])
            # top-2 mask: keep affinities >= second max
            m1 = small.tile([T, 1], f32, tag="r6")
            nc.vector.tensor_reduce(m1[:], aff[:], axis=AX.X, op=ALU.max)
            eq = small.tile([T, E], f32, tag="r7")
            nc.vector.tensor_scalar(eq[:], aff[:], m1[:], None, op0=ALU.is_equal)
            amax = small.tile([T, E], f32, tag="r8")
            nc.vector.tensor_tensor(amax[:], aff[:], eq[:], op=ALU.mult)
            a2 = small.tile([T, E], f32, tag="r9")
            nc.vector.tensor_tensor(a2[:], aff[:], amax[:], op=ALU.subtract)
            m2 = small.tile([T, 1], f32, tag="r10")
            nc.vector.tensor_reduce(m2[:], a2[:], axis=AX.X, op=ALU.max)
            ind = small.tile([T, E], f32, tag="r11")
            nc.vector.tensor_scalar(ind[:], aff[:], m2[:], None, op0=ALU.is_ge)
            smat = small.tile([T, E], f32, tag="r12")
            nc.vector.tensor_tensor(smat[:], aff[:], ind[:], op=ALU.mult)

            # smatT [17,4]: transpose via identity, +1.0 row for shared units
            smT_ps = psmall.tile([E, T], f32, tag="ps")
            nc.tensor.matmul(smT_ps[:], smat[:], id4_s[:], start=True, stop=True)
            smatT = cpool.tile([E + 1, T], f32, tag="smatT")
            nc.vector.memset(smatT[:], 1.0)
            nc.scalar.copy(smatT[0:E, :], smT_ps[:])

            # per-unit replicated scale vectors s_rep[:, u, :] = [128, 4]
            srep = cpool.tile([P, nu, T], f32, tag="srep")
            for u in range(nu):
                sr_ps = psmall.tile([P, T], f32, tag="ps")
                nc.tensor.matmul(
                    sr_ps[:],
                    oh_s[:, u * P : (u + 1) * P],
                    smatT[:],
                    start=True,
                    stop=True,
                )
                nc.scalar.copy(srep[:, u, :], sr_ps[:])

            # ---- main unit loop ----
            acc = [pacc.tile([T, 512], f32, tag=f"acc{b}", name=f"acc{b}") for b in range(4)]
            for u in range(nu):
                wd_t = wdp.tile([P, H], bf16, tag="wd")
                nc.sync.dma_start(wd_t[:], wd_d[u * P : (u + 1) * P, :])

                g_ps = psmall.tile([P, T], f32, tag="ps")
                for ht in range(HT):
                    nc.tensor.matmul(
                        g_ps[:],
                        wg_t[ht][:, u * P : (u + 1) * P],
                        xtb[:, ht, :],
                        start=(ht == 0),
                        stop=(ht == HT - 1),
                    )
                u_ps = psmall.tile([P, T], f32, tag="ps")
                for ht in range(HT):
                    nc.tensor.matmul(
                        u_ps[:],
                        wu_t[ht][:, u * P : (u + 1) * P],
                        xtb[:, ht, :],
                        start=(ht == 0),
                        stop=(ht == HT - 1),
                    )
                sig = small.tile([P, T], f32, tag="sig")
                nc.scalar.activation(sig[:], g_ps[:], AF.Sigmoid)
                sil = small.tile([P, T], f32, tag="sil")
                nc.vector.tensor_tensor(sil[:], sig[:], g_ps[:], op=ALU.mult)
                hh = small.tile([P, T], f32, tag="hh")
                nc.vector.tensor_tensor(hh[:], sil[:], u_ps[:], op=ALU.mult)
                hs = small.tile([P, T], bf16, tag="hs")
                nc.vector.tensor_tensor(hs[:], hh[:], srep[:, u, :], op=ALU.mult)

                for b in range(4):
                    nc.tensor.matmul(
                        acc[b][:],
                        hs[:],
                        wd_t[:, b * 512 : (b + 1) * 512],
                        start=(u == 0),
                        stop=(u == nu - 1),
                    )

            # ---- output ----
            out_s = cpool.tile([T, H], f32, tag="out_s")
            for b in range(4):
                nc.scalar.copy(out_s[:, b * 512 : (b + 1) * 512], acc[b][:])
            nc.sync.dma_start(out_d[:], out_s[:])

    nc.compile()
    return nc


def _get_program(nu: int):
    if nu not in _BUILD_CACHE:
        _BUILD_CACHE[nu] = _build_program(nu)
    return _BUILD_CACHE[nu]


def _host_routing(x: np.ndarray, router_weight: np.ndarray):
    """Mirror of the device routing, used only for the dispatch decision."""
    logits = x.astype(np.float32) @ router_weight.astype(np.float32).T  # [T, E]
    logits -= logits.max(axis=1, keepdims=True)
    ex = np.exp(logits)
    aff = ex / ex.sum(axis=1, keepdims=True)
    idx = np.argsort(-aff, axis=1, kind="stable")[:, :K_TOP]  # [T, 2]
    return idx


def _prepare(
    hidden_states,
    router_weight,
    gate_up_weights,
    down_weights,
    shared_gate_w,
    shared_up_w,
    shared_down_w,
):
    """Host-side dispatch: returns (in_maps, nu)."""
    x = np.asarray(hidden_states, np.float32).reshape(T, H)
    router_weight = np.asarray(router_weight, np.float32)
    gate_up_weights = np.asarray(gate_up_weights, np.float32)
    down_weights = np.asarray(down_weights, np.float32)
    shared_gate_w = np.asarray(shared_gate_w, np.float32)
    shared_up_w = np.asarray(shared_up_w, np.float32)
    shared_down_w = np.asarray(shared_down_w, np.float32)

    # ---- dispatch decision ----
    top_idx = _host_routing(x, router_weight)
    experts = sorted(set(top_idx.ravel().tolist()))

    # flat list of 128-column units: (kind, expert_or_None, col0)
    units = []
    for e in experts:
        for i in range(I_RT // P):
            units.append(("r", e, i * P))
    for j in range(I_SH // P):
        units.append(("s", None, j * P))
    n_real = len(units)
    nu = math.ceil(n_real / NCORES)
    # pad with zero-scale duplicates of the first unit
    units += [("pad",) + units[0][1:]] * (NCORES * nu - n_real)

    # ---- per-core packs ----
    C = nu * P
    xt = np.ascontiguousarray(x.T.reshape(HT, P, T).transpose(1, 0, 2))  # [128,16,4]
    rwt = np.ascontiguousarray(
        router_weight.T.reshape(HT, P, E).transpose(1, 0, 2)
    )  # [128,16,16]
    id4 = np.eye(T, dtype=np.float32)

    in_maps = []
    for c in range(NCORES):
        mine = units[c * nu : (c + 1) * nu]
        wg = np.empty((HT, P, C), BF16)
        wu = np.empty((HT, P, C), BF16)
        wd = np.empty((C, H), BF16)
        oh = np.zeros((E + 1, C), np.float32)
        for u, (kind, e, c0) in enumerate(mine):
            cs = slice(u * P, (u + 1) * P)
            if kind == "s":
                g_blk = shared_gate_w[c0 : c0 + P, :].T  # [2048, 128]
                u_blk = shared_up_w[c0 : c0 + P, :].T
                d_blk = shared_down_w[:, c0 : c0 + P].T  # [128, 2048]
                oh[E, cs] = 1.0
            else:
                g_blk = gate_up_weights[e, :, 0, c0 : c0 + P]  # [2048, 128]
                u_blk = gate_up_weights[e, :, 1, c0 : c0 + P]
                d_blk = down_weights[e, c0 : c0 + P, :]  # [128, 2048]
                if kind == "r":
                    oh[e, cs] = 1.0
            wg[:, :, cs] = g_blk.astype(BF16).reshape(HT, P, P)
            wu[:, :, cs] = u_blk.astype(BF16).reshape(HT, P, P)
            wd[cs, :] = d_blk.astype(BF16)
        in_maps.append(
            {
                "wg": wg,
                "wu": wu,
                "wd": wd,
                "oh": oh,
                "xt": xt,
                "rwt": rwt,
                "id4": id4,
            }
        )
    return in_maps, nu


def kernel(**inputs):
    in_maps, nu = _prepare(**inputs)

    # ---- run on the 8 cores ----
    nc = _get_program(nu)
    from concourse.bass_utils import run_bass_kernel_spmd

    res = run_bass_kernel_spmd(nc, in_maps, list(range(NCORES)))
    global LAST_RESULT
    LAST_RESULT = res
    out = np.zeros((T, H), np.float64)
    for i in range(NCORES):
        out += res.results[i]["out"].astype(np.float64)
    return out.astype(np.float32).reshape(T, 1, H)
